# revision 42
# baseline (speedup 1.0000x reference)
"""Single-head causal attention (B=8, T=2048, C=1024, H=64) on 8 NeuronCores.

Data-parallel over batch: core b computes attention for x[b].

Host-side sharding/relayout (numpy, part of input distribution):
  - xT    [C, T] bf16  : x[b] transposed + cast (contraction dim C on
                         partitions for the projection matmuls).
  - wkv   [128, 8*128] bf16 : [Wk|Wv] packed per C-chunk, pre-permuted so a
                         single contiguous DMA yields per-chunk stationaries.
  - wq    [128, 8*64] bf16 : same for Wq.

Device pipeline per core (engine budget):
  - 5 big x DMAs (quarter-granular, all chunks per instruction) keep the
    Sync sequencer's ~0.7us/DMA issue cost off the critical path.
  - Projections per 512-query block: packed [Wk|Wv] pass -> kvT (kT rows
    0:64, vT 64:128), Wq pass -> qT. bf16, 1 cyc/row on PE.
  - Attention row i: scores S_j = kT_j^T qT_i with exact causal widths,
    software-pipelined AHEAD=3 score tiles in front of the exp (ACT) and
    PV (PE) stages so PE never waits on exp; diagonal 128-col blocks masked
    in-place on Pool; PV accumulates [v|1] @ P in PSUM (row sums free).
  - Next block's projection matmuls + v-naturalization interleave into the
    attention row as PE filler to keep the tensor engine p-state at max.
  - Output: per block, PE transpose back, reciprocal-scale, one DMA.
"""

import numpy as np
import ml_dtypes

import concourse.bass as bass
import concourse.bacc as bacc
import concourse.mybir as mybir
import concourse.tile as tile
from concourse.bass_utils import run_bass_kernel_spmd
from concourse.masks import make_identity

B = 8
T, C, H = 2048, 1024, 64
P = 128
NCHUNK = C // P  # 8
QT = 512         # query-block width
NQ = T // QT     # 4
NT = T // P      # 16
H1 = H + 1
AHEAD = 3        # score tiles issued ahead of PV
f32 = mybir.dt.float32
bf16 = mybir.dt.bfloat16
EXP = mybir.ActivationFunctionType.Exp


def build_nc() -> bass.Bass:
    nc = bacc.Bacc("TRN2", target_bir_lowering=False, debug=False)
    xT = nc.dram_tensor("xT", [C, T], bf16, kind="ExternalInput")
    wkv = nc.dram_tensor("wkv", [P, NCHUNK * P], bf16, kind="ExternalInput")
    wq = nc.dram_tensor("wq", [P, NCHUNK * H], bf16, kind="ExternalInput")
    out = nc.dram_tensor("out", [T, H], f32, kind="ExternalOutput")

    with tile.TileContext(nc) as tc:
        with (
            tc.tile_pool(name="const", bufs=1) as constp,
            tc.tile_pool(name="w", bufs=1) as wp,
            tc.tile_pool(name="xt", bufs=1) as xtp,
            tc.tile_pool(name="qkv", bufs=1) as qkvp,
            tc.tile_pool(name="pt", bufs=5) as ptp,
            tc.tile_pool(name="fin", bufs=2) as finp,
            tc.tile_pool(name="pss", bufs=5, space="PSUM") as pss,
            tc.tile_pool(name="pspo", bufs=1, space="PSUM") as pspo,
            tc.tile_pool(name="psproj", bufs=2, space="PSUM") as psproj,
        ):
            identf = constp.tile([P, P], f32, tag="identf")
            make_identity(nc, identf)
            identb = constp.tile([P, P], bf16, tag="identb")
            make_identity(nc, identb)

            # x: one [128, NCHUNK*T] tile, chunk-major; weights first, then
            # the first quarter in chunk-pairs (earliest-consumed first)
            xt = xtp.tile([P, NCHUNK * T], bf16, tag="xt")
            xt3 = xt.rearrange("p (c t) -> p c t", t=T)
            src3 = xT.rearrange("(c p) t -> p c t", p=P)
            wkv_sb = wp.tile([P, NCHUNK * P], bf16, tag="wkv")
            wq_sb = wp.tile([P, NCHUNK * H], bf16, tag="wq")
            nc.sync.dma_start(out=wkv_sb, in_=wkv[:, :])
            nc.sync.dma_start(out=xt3[:, 0:2, 0:QT], in_=src3[:, 0:2, 0:QT])
            nc.sync.dma_start(out=wq_sb, in_=wq[:, :])
            for cp in range(1, 4):
                nc.sync.dma_start(out=xt3[:, 2 * cp : 2 * cp + 2, 0:QT],
                                  in_=src3[:, 2 * cp : 2 * cp + 2, 0:QT])
            for q4 in range(1, NQ):
                nc.sync.dma_start(out=xt3[:, :, q4 * QT : (q4 + 1) * QT],
                                  in_=src3[:, :, q4 * QT : (q4 + 1) * QT])

            def xchunk(c, ts):
                return xt[:, c * T + ts.start : c * T + ts.stop]

            kvT = qkvp.tile([P, T], bf16, tag="kvT")   # kT 0:64, vT 64:128
            qT = qkvp.tile([H, T], bf16, tag="qT")
            vsb = qkvp.tile([P, NT * H1], bf16, tag="vsb")
            ones = constp.tile([P, NT], bf16, tag="ones")
            nc.vector.memset(ones, 1.0)
            nc.vector.tensor_copy(
                vsb.rearrange("p (t w) -> p t w", w=H1)[:, :, H:H1],
                ones.unsqueeze(2))

            def emit_proj(i):
                """Projection + v-naturalization ops for block i, as thunks."""
                ts = slice(i * QT, (i + 1) * QT)
                items = []
                pkv = psproj.tile([P, QT], f32, tag="psp", name=f"pkv{i}")
                pq = psproj.tile([H, QT], f32, tag="psp", name=f"pq{i}")

                def kvmm(c, pkv=pkv, ts=ts):
                    nc.tensor.matmul(pkv, wkv_sb[:, c * P : (c + 1) * P],
                                     xchunk(c, ts),
                                     start=(c == 0), stop=(c == NCHUNK - 1))

                def qmm(c, pq=pq, ts=ts):
                    nc.tensor.matmul(pq, wq_sb[:, c * H : (c + 1) * H],
                                     xchunk(c, ts),
                                     start=(c == 0), stop=(c == NCHUNK - 1))

                if i == 0:
                    # consume each arriving chunk-pair twice (kv then q) so
                    # the PE is not paced by the x DMA during block 0
                    for cp in range(NCHUNK // 2):
                        for c in (2 * cp, 2 * cp + 1):
                            items.append(lambda c=c: kvmm(c))
                        for c in (2 * cp, 2 * cp + 1):
                            items.append(lambda c=c: qmm(c))
                    items.append(lambda pkv=pkv: nc.vector.tensor_copy(
                        kvT[:, ts], pkv))
                    items.append(lambda pq=pq: nc.vector.tensor_copy(
                        qT[:, ts], pq))
                else:
                    for c in range(NCHUNK):
                        items.append(lambda c=c: kvmm(c))
                    items.append(lambda pkv=pkv: nc.vector.tensor_copy(
                        kvT[:, ts], pkv))
                    for c in range(NCHUNK):
                        items.append(lambda c=c: qmm(c))
                    items.append(lambda pq=pq: nc.vector.tensor_copy(
                        qT[:, ts], pq))
                for t in range(i * 4, i * 4 + 4):
                    pvt = psproj.tile([P, H], bf16, tag="psp", name=f"pvt{t}")
                    items.append(lambda t=t, pvt=pvt: nc.tensor.transpose(
                        pvt, kvT[H:P, t * P : (t + 1) * P], identb[H:P, H:P]))
                    items.append(lambda t=t, pvt=pvt: nc.vector.tensor_copy(
                        vsb[:, t * H1 : t * H1 + H], pvt))
                return items

            def emit_out(i):
                """Normalize + store for block i, as thunks (popped during
                row i+1 so the out stage overlaps the next attention row)."""
                ts = slice(i * QT, (i + 1) * QT)
                items = []
                ot = finp.tile([H1, QT], f32, tag="ot", name=f"ot{i}")
                last = i == NQ - 1
                po = po_ref[i]
                ob = finp.tile([P, 4 * H], f32, tag="ob", name=f"ob{i}")
                if not last:
                    items.append(lambda po=po, ot=ot:
                                 nc.vector.tensor_copy(ot, po))
                for b in range(QT // P):
                    if last:  # per-sub-block chains so the tail pipelines
                        items.append(lambda po=po, ot=ot, b=b:
                                     nc.vector.tensor_copy(
                                         ot[:, b * P : (b + 1) * P],
                                         po[:, b * P : (b + 1) * P]))
                    pot = psproj.tile([P, H1], f32, tag="psp", name=f"o{i}{b}")
                    items.append(lambda ot=ot, b=b, pot=pot:
                                 nc.tensor.transpose(
                                     pot, ot[:, b * P : (b + 1) * P],
                                     identf[:H1, :H1]))
                    rcp = finp.tile([P, 1], f32, tag="rcp")
                    items.append(lambda pot=pot, rcp=rcp:
                                 nc.vector.reciprocal(rcp, pot[:, H : H + 1]))
                    if last:  # ACT is idle at the tail; keep it out of the
                        # exp stream otherwise (in-order engine)
                        items.append(lambda pot=pot, rcp=rcp, b=b, ob=ob:
                                     nc.scalar.mul(
                                         ob[:, b * H : (b + 1) * H],
                                         pot[:, 0:H], rcp))
                    else:
                        items.append(lambda pot=pot, rcp=rcp, b=b, ob=ob:
                                     nc.vector.tensor_scalar_mul(
                                         ob[:, b * H : (b + 1) * H],
                                         pot[:, 0:H], rcp))
                    if last:  # fine-grained tail: store each 128-row block
                        items.append(lambda b=b, i=i, ob=ob:
                                     nc.sync.dma_start(
                                         out=out[i * QT + b * P :
                                                 i * QT + (b + 1) * P, :],
                                         in_=ob[:, b * H : (b + 1) * H]))
                if not last:
                    items.append(lambda i=i, ts=ts, ob=ob: nc.sync.dma_start(
                        out=out[ts, :].rearrange("(b p) h -> p b h", p=P),
                        in_=ob.rearrange("p (b h) -> p b h", h=H)))
                return items

            # PE warm-up: ramp the tensor-engine p-state under the DMA
            # window with dependency-free transposes of the identity
            for wi in range(14):
                pw = pss.tile([P, P], f32, tag="pss", name=f"warm{wi}")
                nc.tensor.transpose(pw, identf, identf)

            po_ref = {}

            # block 0 projections run up front
            for it in emit_proj(0):
                it()

            for i in range(NQ):
                ts = slice(i * QT, (i + 1) * QT)
                q_i = qT[:, ts]
                js = list(range(4 * i + 4))
                nj = len(js)
                out_items = emit_out(i - 1) if i > 0 else []
                proj_items = emit_proj(i + 1) if i + 1 < NQ else []
                # zip-merge so DVE-heavy out items spread between PE-heavy
                # projection items instead of bursting
                filler = []
                while out_items or proj_items:
                    if proj_items:
                        filler.append(proj_items.pop(0))
                    if proj_items:
                        filler.append(proj_items.pop(0))
                    if out_items:
                        filler.append(out_items.pop(0))
                per_step = -(-len(filler) // nj) if filler else 0

                pstiles = {}
                pttiles = {}

                def sub_of(j, i=i):
                    return max(j - 4 * i, 0) * P

                def scores(j, i=i, q_i=q_i):
                    sub = sub_of(j)
                    ps = pss.tile([P, QT], f32, tag="pss", name=f"s{i}_{j}")
                    pstiles[j] = ps
                    nc.tensor.matmul(ps[:, sub:QT],
                                     kvT[0:H, j * P : (j + 1) * P],
                                     q_i[:, sub:QT], start=True, stop=True)

                for j in js[:AHEAD]:
                    scores(j)

                po = pspo.tile([H1, QT], f32, tag="po", name=f"po{i}")
                last_row = i == NQ - 1
                if last_row:
                    # final block: dissolve the out stage into the j-loop —
                    # po sub-block b is final right after diagonal PV 4i+b
                    ot3 = finp.tile([H1, QT], f32, tag="ot", name="ot3")
                    ob3 = finp.tile([P, 4 * H], f32, tag="ob", name="ob3")

                    def out3_chain(b, i=i):
                        pot = psproj.tile([P, H1], f32, tag="psp",
                                          name=f"o3{b}")
                        nc.tensor.transpose(pot, ot3[:, b * P : (b + 1) * P],
                                            identf[:H1, :H1])
                        rcp = finp.tile([P, 1], f32, tag="rcp")
                        nc.vector.reciprocal(rcp, pot[:, H : H + 1])
                        nc.scalar.mul(ob3[:, b * H : (b + 1) * H],
                                      pot[:, 0:H], rcp)
                        nc.sync.dma_start(
                            out=out[i * QT + b * P : i * QT + (b + 1) * P, :],
                            in_=ob3[:, b * H : (b + 1) * H])
                for k, j in enumerate(js):
                    sub = sub_of(j)
                    ps = pstiles.pop(j)
                    pt = ptp.tile([P, QT], bf16, tag="pt", name=f"p{i}_{j}")
                    nc.scalar.activation(pt[:, sub:QT], ps[:, sub:QT],
                                         EXP, scale=0.125)
                    if j >= 4 * i:  # diagonal: mask leading 128-col block
                        nc.gpsimd.affine_select(
                            out=pt[:, sub : sub + P],
                            in_=pt[:, sub : sub + P],
                            pattern=[[1, P]],
                            compare_op=mybir.AluOpType.is_ge, fill=0.0,
                            base=0, channel_multiplier=-1)
                    if k + AHEAD < nj:
                        scores(js[k + AHEAD])
                    nc.tensor.matmul(po[:, sub:QT],
                                     vsb[:, j * H1 : (j + 1) * H1],
                                     pt[:, sub:QT],
                                     start=(j == 0), stop=(j == js[-1]))
                    if last_row and j >= 4 * i:
                        b = j - 4 * i
                        nc.vector.tensor_copy(ot3[:, b * P : (b + 1) * P],
                                              po[:, b * P : (b + 1) * P])
                        if b >= 1:
                            out3_chain(b - 1)
                    for _ in range(per_step):
                        if filler:
                            filler.pop(0)()

                while filler:
                    filler.pop(0)()
                po_ref[i] = po
                if last_row:
                    out3_chain(3)
    nc.compile()
    return nc


_NC_CACHE = None


def _get_nc():
    global _NC_CACHE
    if _NC_CACHE is None:
        _NC_CACHE = build_nc()
    return _NC_CACHE


def run(in_maps, trace=False, **kw):
    nc = _get_nc()
    return run_bass_kernel_spmd(nc, in_maps, core_ids=list(range(B)),
                                trace=trace, **kw)


def _prep_weights(Wq, Wk, Wv):
    bf = ml_dtypes.bfloat16
    wkv_nat = np.concatenate([Wk, Wv], axis=1)  # [C, 128]
    wkv = np.ascontiguousarray(
        wkv_nat.reshape(NCHUNK, P, P).transpose(1, 0, 2).reshape(P, NCHUNK * P)
    ).astype(bf)
    wq = np.ascontiguousarray(
        Wq.reshape(NCHUNK, P, H).transpose(1, 0, 2).reshape(P, NCHUNK * H)
    ).astype(bf)
    return wkv, wq


def make_in_maps(x, Wq, Wk, Wv):
    bf = ml_dtypes.bfloat16
    x = np.asarray(x, dtype=np.float32)
    wkv, wq = _prep_weights(np.asarray(Wq, dtype=np.float32),
                            np.asarray(Wk, dtype=np.float32),
                            np.asarray(Wv, dtype=np.float32))
    return [
        {"xT": np.ascontiguousarray(x[b].T).astype(bf), "wkv": wkv, "wq": wq}
        for b in range(B)
    ]


def kernel(x, Wq, Wk, Wv):
    res = run(make_in_maps(x, Wq, Wk, Wv))
    return np.stack([res.results[b]["out"] for b in range(B)], axis=0)
